# revision 19
# baseline (speedup 1.0000x reference)
"""Trainium2 Bass kernel for nn_AuxEntTypeTower (BCE-with-logits aux loss).

Computes, given feature [4,1024,512], target [4,1024,8192], mask [4,1024],
head_w [8192,512], head_b [8192], alpha []:
    logits = feature @ head_w.T + head_b                      # [4,1024,8192]
    prev   = concat([target[:, :1], sigmoid(logits[:, :-1])], axis=1)
    st     = (1-alpha)*target + alpha*prev
    per    = softplus(logits) - logits*st
    loss   = sum(per * mask[...,None]) / max(sum(mask), 1)
Returns (logits, loss).

Sharding: tensor-parallel over the type dim T=8192 across 8 cores (1024
types per core). Each core computes its logits chunk plus three per-token
partial reductions; the host combines them into the scalar loss:
    r1[tok] = sum_t softplus(-x)          (softplus(x) = x + softplus(-x))
    r2[tok] = sum_t x*target
    r3[tok] = sum_t x*sigmoid(x_prev_row) (prev-row shift, 0 at batch starts)
Device transcendentals use only the natural_log_exp table set:
    en = exp(-x); spn = ln(1+en) = softplus(-x); sg = exp(-spn) = sigmoid(x).
"""

import types

import numpy as np

import bass_rust as _bass_rust
import concourse.bacc as bacc
import concourse.bass as bass
import concourse.mybir as mybir
import concourse.tile as tile
from concourse.bass_utils import run_bass_kernel_spmd
from concourse.hw_specs import get_activation_tables

B, L, H, T = 4, 1024, 512, 8192
NCORES = 8
TC = T // NCORES          # 1024 types per core
TOK = B * L               # 4096 tokens
P = 128                   # partitions per tile
NTILES = TOK // P         # 32 token tiles per core
KT = H // P               # 4 contraction sub-tiles
NH = TC // 512            # 2 matmul n-halves per tile
BATCH_TILES = L // P      # 8 tiles per batch sequence

F32 = mybir.dt.float32
F32R = mybir.dt.float32r

_CACHE = {}


def _build_nc(repeat=1):
    """repeat>1 re-runs the whole pipeline (for marginal-cost timing)."""
    nc = bacc.Bacc("TRN2", target_bir_lowering=False, debug=False,
                   num_devices=NCORES)

    featT = nc.declare_dram_parameter("featT", [NTILES, P, KT, P], F32,
                                      isOutput=False)
    wT = nc.declare_dram_parameter("wT", [P, KT, TC], F32, isOutput=False)
    bias = nc.declare_dram_parameter("bias", [1, TC], F32, isOutput=False)
    tgt = nc.declare_dram_parameter("tgt", [TOK, TC], F32, isOutput=False)
    logits = nc.declare_dram_parameter("logits", [TOK, TC], F32, isOutput=True)
    racc_d = nc.declare_dram_parameter("racc", [P, 3 * NTILES], F32,
                                       isOutput=True)

    AF = mybir.ActivationFunctionType
    OP = mybir.AluOpType

    with tile.TileContext(nc) as tc:
        # shift matrices: S moves row p -> p+1 within a tile (superdiagonal),
        # E moves prev-tile row 127 -> row 0 (for the cross-tile boundary)
        s_np = np.zeros((P, P), np.float32)
        for m in range(1, P):
            s_np[m - 1, m] = 1.0
        e_np = np.zeros((P, P), np.float32)
        e_np[P - 1, 0] = 1.0
        s_dram = nc.inline_tensor(s_np, name="shiftS")
        e_dram = nc.inline_tensor(e_np, name="shiftE")

        with (
            tc.tile_pool(name="consts", bufs=1) as consts,
            tc.tile_pool(name="feat", bufs=6) as featp,
            tc.tile_pool(name="featr", bufs=6) as featrp,
            tc.tile_pool(name="tgtp", bufs=6) as tgtp,
            tc.tile_pool(name="xps", bufs=2, space="PSUM") as xps,
            tc.tile_pool(name="sgps", bufs=2, space="PSUM") as sgps,
            tc.tile_pool(name="xsb", bufs=4) as xsbp,
            tc.tile_pool(name="en", bufs=3) as enp,
            tc.tile_pool(name="spn", bufs=3) as spnp,
            tc.tile_pool(name="sg", bufs=4) as sgp,
            tc.tile_pool(name="trash", bufs=3) as trashp,
        ):
            s_f32 = consts.tile([P, P], F32)
            nc.sync.dma_start(out=s_f32, in_=s_dram[:])
            s_r = consts.tile([P, P], F32R)
            nc.gpsimd.tensor_copy(out=s_r, in_=s_f32)
            e_f32 = consts.tile([P, P], F32)
            nc.sync.dma_start(out=e_f32, in_=e_dram[:])
            e_r = consts.tile([P, P], F32R)
            nc.gpsimd.tensor_copy(out=e_r, in_=e_f32)
            # fp32r operands: HWDGE loads raw fp32, a DVE copy rounds to fp32r
            # (SWDGE cast-DMAs are much slower than HWDGE + engine cast)
            wT_f32 = consts.tile([P, KT, TC], F32)
            nc.sync.dma_start(out=wT_f32, in_=wT[:])
            wT_sb = consts.tile([P, KT, TC], F32R)
            nc.gpsimd.tensor_copy(out=wT_sb, in_=wT_f32)
            # head_b broadcast across all 128 partitions (exact fp32 bias add)
            bias_sb = consts.tile([P, TC], F32)
            bias_ap = bias[:]
            bias_bcast = bass.AP(tensor=bias_ap.tensor, offset=bias_ap.offset,
                                 ap=[[0, P], [1, TC]])
            nc.sync.dma_start(out=bias_sb, in_=bias_bcast)
            racc = consts.tile([P, 3 * NTILES], F32)

            for _rep in range(repeat):
              sg_prev = None
              for i in range(NTILES):
                feat_f32 = featp.tile([P, KT, P], F32)
                nc.sync.dma_start(out=feat_f32, in_=featT[i])
                feat_t = featrp.tile([P, KT, P], F32R)
                nc.gpsimd.tensor_copy(out=feat_t, in_=feat_f32)
                tgt_t = tgtp.tile([P, TC], F32)
                nc.sync.dma_start(out=tgt_t, in_=tgt[i * P:(i + 1) * P, :])

                x_ps = xps.tile([P, TC], F32)
                for nh in range(NH):
                    csl = bass.ts(nh, 512)
                    for k in range(KT):
                        nc.tensor.matmul(
                            out=x_ps[:, csl],
                            lhsT=feat_t[:, k, :],
                            rhs=wT_sb[:, k, csl],
                            start=(k == 0),
                            stop=(k == KT - 1),
                        )

                # x = psum + bias  (exact fp32), lands in SBUF for DMA/reads
                x_sb = xsbp.tile([P, TC], F32)
                nc.vector.tensor_add(out=x_sb, in0=x_ps, in1=bias_sb)
                nc.sync.dma_start(out=logits[i * P:(i + 1) * P, :], in_=x_sb)

                # ACT chain (single table set): en=exp(-x), spn=ln(1+en),
                # sg=exp(-spn)=sigmoid(x). accum(spn) -> r1 column.
                en_t = enp.tile([P, TC], F32)
                nc.scalar.activation(out=en_t, in_=x_sb, func=AF.Exp,
                                     scale=-1.0)
                spn_t = spnp.tile([P, TC], F32)
                nc.scalar.activation(out=spn_t, in_=en_t, func=AF.Ln,
                                     bias=1.0, accum_out=racc[:, i:i + 1])
                # sigmoid, written as fp32r so the PE can consume it
                sg_t = sgp.tile([P, TC], F32R)
                nc.scalar.activation(out=sg_t, in_=spn_t, func=AF.Exp,
                                     scale=-1.0)

                # prev-row shift of sigmoid on the PE: sgs = S.T@sg (+E.T@prev)
                sgs_t = sgps.tile([P, TC], F32)
                first = i % BATCH_TILES == 0
                for nh in range(NH):
                    csl = bass.ts(nh, 512)
                    nc.tensor.matmul(out=sgs_t[:, csl], lhsT=s_r,
                                     rhs=sg_t[:, csl],
                                     start=True, stop=first)
                    if not first:
                        nc.tensor.matmul(out=sgs_t[:, csl], lhsT=e_r,
                                         rhs=sg_prev[:, csl],
                                         start=False, stop=True)
                sg_prev = sg_t

                # fused multiply+reduce: r2 = sum x*t, r3 = sum x*sg_shift
                # (scalar_tensor_tensor: out=(in0*1.0)*in1, accum=sum(out))
                tr = trashp.tile([P, TC], F32)
                nc.vector.scalar_tensor_tensor(
                    out=tr, in0=x_sb, scalar=1.0, in1=tgt_t,
                    op0=OP.mult, op1=OP.mult,
                    accum_out=racc[:, NTILES + i:NTILES + i + 1])
                tr2 = trashp.tile([P, TC], F32)
                nc.vector.scalar_tensor_tensor(
                    out=tr2, in0=x_sb, scalar=1.0, in1=sgs_t,
                    op0=OP.mult, op1=OP.mult,
                    accum_out=racc[:, 2 * NTILES + i:2 * NTILES + i + 1])

            nc.sync.dma_start(out=racc_d[:], in_=racc)

    # All our transcendentals (Exp, Ln) live in natural_log_exp_and_others.
    # The default table-load pass maps Exp->exp_and_others and
    # Ln->natural_log, inserting a ~2.7us table swap before nearly every
    # activation. Restrict the pass to the one set that covers both.
    def _patched_insert_act_table_loads(self):
        has_activation = any(
            isinstance(i, mybir.InstActivation)
            for b in self.main_func.blocks for i in b.instructions)
        if not has_activation:
            return
        keep = "natural_log_exp_and_others"
        tables = [(n, (s if n == keep else set()))
                  for n, s in get_activation_tables(self.m.arch).items()]
        _bass_rust.insert_act_table_loads(self, tables)

    nc.insert_act_table_loads = types.MethodType(
        _patched_insert_act_table_loads, nc)

    nc.compile()
    return nc


def _get_nc(repeat=1):
    key = ("nc", repeat)
    if key not in _CACHE:
        _CACHE[key] = _build_nc(repeat)
    return _CACHE[key]


def make_in_maps(feature, target, head_w, head_b):
    """Host-side sharding: returns per-core input dicts."""
    feature = np.ascontiguousarray(np.asarray(feature, dtype=np.float32))
    target = np.asarray(target, dtype=np.float32)
    head_w = np.asarray(head_w, dtype=np.float32)
    head_b = np.asarray(head_b, dtype=np.float32)

    # featT tiles: [NTILES, P(p=h%128... p is h within k), KT, P(tokens)]
    # featT_tiles[i, p, k, f] = feature_flat[i*128+f, k*128+p]
    ff = feature.reshape(TOK, H)
    featT_tiles = np.ascontiguousarray(
        ff.reshape(NTILES, P, KT, P).transpose(0, 3, 2, 1))

    tflat = target.reshape(TOK, T)
    in_maps = []
    for c in range(NCORES):
        hw_c = head_w[c * TC:(c + 1) * TC, :]       # [TC, H]
        wT_c = np.ascontiguousarray(
            hw_c.reshape(TC, KT, P).transpose(2, 1, 0))  # [P, KT, TC]
        b_c = np.ascontiguousarray(head_b[c * TC:(c + 1) * TC].reshape(1, TC))
        tgt_c = np.ascontiguousarray(tflat[:, c * TC:(c + 1) * TC])
        in_maps.append({"featT": featT_tiles, "wT": wT_c, "bias": b_c,
                        "tgt": tgt_c})
    return in_maps


def combine_outputs(results, target, mask, alpha):
    """Host-side gather: assemble logits + scalar loss from per-core outs."""
    target = np.asarray(target, dtype=np.float32)
    mask_f = np.asarray(mask).reshape(TOK).astype(np.float64)
    alpha = float(np.asarray(alpha))

    logits = np.empty((TOK, T), dtype=np.float32)
    r1 = np.zeros(TOK, dtype=np.float64)
    r2 = np.zeros(TOK, dtype=np.float64)
    r3 = np.zeros(TOK, dtype=np.float64)
    for c in range(NCORES):
        logits[:, c * TC:(c + 1) * TC] = results[c]["logits"]
        racc = results[c]["racc"].astype(np.float64)  # [P, 3*NTILES]
        # column i holds tile i's [128] partial; token = i*128 + p
        r1 += racc[:, 0:NTILES].T.reshape(TOK)
        r2 += racc[:, NTILES:2 * NTILES].T.reshape(TOK)
        r3 += racc[:, 2 * NTILES:3 * NTILES].T.reshape(TOK)

    rowsum_x = logits.sum(axis=1, dtype=np.float64)
    S1 = float(np.dot(mask_f, rowsum_x + r1))   # sum softplus(x)
    S2 = float(np.dot(mask_f, r2))              # sum x*target
    S3 = float(np.dot(mask_f, r3))              # sum x*sigmoid(prev), l>=1
    # l == 0 rows: prev = target[:, 0]
    lg = logits.reshape(B, L, T)
    tg = target.reshape(B, L, T)
    for b in range(B):
        S3 += mask_f[b * L] * float(
            np.dot(lg[b, 0].astype(np.float64), tg[b, 0].astype(np.float64)))

    num = S1 - (1.0 - alpha) * S2 - alpha * S3
    denom = max(mask_f.sum(), 1.0)
    loss = np.float32(num / denom)
    return lg, loss


def kernel(feature, target, mask, head_w, head_b, alpha):
    nc = _get_nc()
    in_maps = make_in_maps(feature, target, head_w, head_b)
    res = run_bass_kernel_spmd(nc, in_maps, list(range(NCORES)))
    logits, loss = combine_outputs(res.results, target, mask, alpha)
    return logits, loss


if __name__ == "__main__":
    # quick self-run with random data
    rng = np.random.default_rng(0)
    feature = rng.standard_normal((B, L, H), dtype=np.float32)
    target = rng.random((B, L, T), dtype=np.float32)
    mask = np.ones((B, L), dtype=bool)
    head_w = (rng.standard_normal((T, H), dtype=np.float32) / np.sqrt(H)).astype(np.float32)
    head_b = np.zeros((T,), dtype=np.float32)
    alpha = np.float32(0.37)
    lg, loss = kernel(feature=feature, target=target, mask=mask,
                      head_w=head_w, head_b=head_b, alpha=alpha)
    print("logits", lg.shape, lg.dtype, "loss", loss)


# revision 20
# speedup vs baseline: 1.9344x; 1.9344x over previous
"""Trainium2 Bass kernel for nn_AuxEntTypeTower (BCE-with-logits aux loss).

Computes, given feature [4,1024,512], target [4,1024,8192], mask [4,1024],
head_w [8192,512], head_b [8192], alpha []:
    logits = feature @ head_w.T + head_b                      # [4,1024,8192]
    prev   = concat([target[:, :1], sigmoid(logits[:, :-1])], axis=1)
    st     = (1-alpha)*target + alpha*prev
    per    = softplus(logits) - logits*st
    loss   = sum(per * mask[...,None]) / max(sum(mask), 1)
Returns (logits, loss).

Sharding: tensor-parallel over the type dim T=8192 across 8 cores (1024
types per core). Each core computes its logits chunk plus three per-token
partial reductions; the host combines them into the scalar loss:
    r1[tok] = sum_t softplus(-x)          (softplus(x) = x + softplus(-x))
    r2[tok] = sum_t x*target
    r3[tok] = sum_t x*sigmoid(x_prev_row) (prev-row shift, 0 at batch starts)
Device transcendentals use only the natural_log_exp table set:
    en = exp(-x); spn = ln(1+en) = softplus(-x); sg = exp(-spn) = sigmoid(x).
"""

import types

import numpy as np

import bass_rust as _bass_rust
import concourse.bacc as bacc
import concourse.bass as bass
import concourse.mybir as mybir
import concourse.tile as tile
from concourse.bass_utils import run_bass_kernel_spmd
from concourse.hw_specs import get_activation_tables

B, L, H, T = 4, 1024, 512, 8192
NCORES = 8
TC = T // NCORES          # 1024 types per core
TOK = B * L               # 4096 tokens
P = 128                   # partitions per tile
NTILES = TOK // P         # 32 token tiles per core
KT = H // P               # 4 contraction sub-tiles
NH = TC // 512            # 2 matmul n-halves per tile
BATCH_TILES = L // P      # 8 tiles per batch sequence

F32 = mybir.dt.float32
F32R = mybir.dt.float32r

_CACHE = {}


def _build_nc(repeat=1):
    """repeat>1 re-runs the whole pipeline (for marginal-cost timing)."""
    nc = bacc.Bacc("TRN2", target_bir_lowering=False, debug=False,
                   num_devices=NCORES)

    featT = nc.declare_dram_parameter("featT", [NTILES, P, KT, P], F32,
                                      isOutput=False)
    wT = nc.declare_dram_parameter("wT", [P, KT, TC], F32, isOutput=False)
    bias = nc.declare_dram_parameter("bias", [1, TC], F32, isOutput=False)
    tgt = nc.declare_dram_parameter("tgt", [TOK, TC], F32, isOutput=False)
    logits = nc.declare_dram_parameter("logits", [TOK, TC], F32, isOutput=True)
    racc_d = nc.declare_dram_parameter("racc", [P, 3 * NTILES], F32,
                                       isOutput=True)

    AF = mybir.ActivationFunctionType
    OP = mybir.AluOpType

    with tile.TileContext(nc) as tc:
        # shift matrices: S moves row p -> p+1 within a tile (superdiagonal),
        # E moves prev-tile row 127 -> row 0 (for the cross-tile boundary)
        s_np = np.zeros((P, P), np.float32)
        for m in range(1, P):
            s_np[m - 1, m] = 1.0
        e_np = np.zeros((P, P), np.float32)
        e_np[P - 1, 0] = 1.0
        s_dram = nc.inline_tensor(s_np, name="shiftS")
        e_dram = nc.inline_tensor(e_np, name="shiftE")

        with (
            tc.tile_pool(name="consts", bufs=1) as consts,
            tc.tile_pool(name="feat", bufs=4) as featp,
            tc.tile_pool(name="featr", bufs=4) as featrp,
            tc.tile_pool(name="tgtp", bufs=4) as tgtp,
            tc.tile_pool(name="xps", bufs=2, space="PSUM") as xps,
            tc.tile_pool(name="sgps", bufs=2, space="PSUM") as sgps,
            tc.tile_pool(name="xsb", bufs=3) as xsbp,
            tc.tile_pool(name="en", bufs=2) as enp,
            tc.tile_pool(name="spn", bufs=2) as spnp,
            tc.tile_pool(name="sg", bufs=3) as sgp,
            tc.tile_pool(name="trash", bufs=2) as trashp,
        ):
            s_f32 = consts.tile([P, P], F32)
            nc.sync.dma_start(out=s_f32, in_=s_dram[:])
            s_r = consts.tile([P, P], F32R)
            nc.gpsimd.tensor_copy(out=s_r, in_=s_f32)
            e_f32 = consts.tile([P, P], F32)
            nc.sync.dma_start(out=e_f32, in_=e_dram[:])
            e_r = consts.tile([P, P], F32R)
            nc.gpsimd.tensor_copy(out=e_r, in_=e_f32)
            # fp32r operands: HWDGE loads raw fp32, a DVE copy rounds to fp32r
            # (SWDGE cast-DMAs are much slower than HWDGE + engine cast)
            wT_f32 = consts.tile([P, KT, TC], F32)
            nc.sync.dma_start(out=wT_f32, in_=wT[:])
            wT_sb = consts.tile([P, KT, TC], F32R)
            nc.gpsimd.tensor_copy(out=wT_sb, in_=wT_f32)
            # head_b broadcast across all 128 partitions (exact fp32 bias add)
            bias_sb = consts.tile([P, TC], F32)
            bias_ap = bias[:]
            bias_bcast = bass.AP(tensor=bias_ap.tensor, offset=bias_ap.offset,
                                 ap=[[0, P], [1, TC]])
            nc.sync.dma_start(out=bias_sb, in_=bias_bcast)
            racc = consts.tile([P, 3 * NTILES], F32)

            for _rep in range(repeat):
              sg_prev = None
              for i in range(NTILES):
                feat_f32 = featp.tile([P, KT, P], F32)
                nc.sync.dma_start(out=feat_f32, in_=featT[i])
                feat_t = featrp.tile([P, KT, P], F32R)
                nc.gpsimd.tensor_copy(out=feat_t, in_=feat_f32)
                tgt_t = tgtp.tile([P, TC], F32)
                nc.sync.dma_start(out=tgt_t, in_=tgt[i * P:(i + 1) * P, :])

                x_ps = xps.tile([P, TC], F32)
                for nh in range(NH):
                    csl = bass.ts(nh, 512)
                    for k in range(KT):
                        nc.tensor.matmul(
                            out=x_ps[:, csl],
                            lhsT=feat_t[:, k, :],
                            rhs=wT_sb[:, k, csl],
                            start=(k == 0),
                            stop=(k == KT - 1),
                        )

                # x = psum + bias  (exact fp32), lands in SBUF for DMA/reads
                x_sb = xsbp.tile([P, TC], F32)
                nc.vector.tensor_add(out=x_sb, in0=x_ps, in1=bias_sb)
                nc.sync.dma_start(out=logits[i * P:(i + 1) * P, :], in_=x_sb)

                # ACT chain (single table set): en=exp(-x), spn=ln(1+en),
                # sg=exp(-spn)=sigmoid(x). accum(spn) -> r1 column.
                en_t = enp.tile([P, TC], F32)
                nc.scalar.activation(out=en_t, in_=x_sb, func=AF.Exp,
                                     scale=-1.0)
                spn_t = spnp.tile([P, TC], F32)
                nc.scalar.activation(out=spn_t, in_=en_t, func=AF.Ln,
                                     bias=1.0, accum_out=racc[:, i:i + 1])
                # sigmoid, written as fp32r so the PE can consume it
                sg_t = sgp.tile([P, TC], F32R)
                nc.scalar.activation(out=sg_t, in_=spn_t, func=AF.Exp,
                                     scale=-1.0)

                # prev-row shift of sigmoid on the PE: sgs = S.T@sg (+E.T@prev)
                sgs_t = sgps.tile([P, TC], F32)
                first = i % BATCH_TILES == 0
                for nh in range(NH):
                    csl = bass.ts(nh, 512)
                    nc.tensor.matmul(out=sgs_t[:, csl], lhsT=s_r,
                                     rhs=sg_t[:, csl],
                                     start=True, stop=first)
                    if not first:
                        nc.tensor.matmul(out=sgs_t[:, csl], lhsT=e_r,
                                         rhs=sg_prev[:, csl],
                                         start=False, stop=True)
                sg_prev = sg_t

                # fused multiply+reduce: r2 = sum x*t, r3 = sum x*sg_shift
                # (scalar_tensor_tensor: out=(in0*1.0)*in1, accum=sum(out))
                tr = trashp.tile([P, TC], F32)
                nc.vector.scalar_tensor_tensor(
                    out=tr, in0=x_sb, scalar=1.0, in1=tgt_t,
                    op0=OP.mult, op1=OP.mult,
                    accum_out=racc[:, NTILES + i:NTILES + i + 1])
                tr2 = trashp.tile([P, TC], F32)
                nc.vector.scalar_tensor_tensor(
                    out=tr2, in0=x_sb, scalar=1.0, in1=sgs_t,
                    op0=OP.mult, op1=OP.mult,
                    accum_out=racc[:, 2 * NTILES + i:2 * NTILES + i + 1])

            nc.sync.dma_start(out=racc_d[:], in_=racc)

    # All our transcendentals (Exp, Ln) live in natural_log_exp_and_others.
    # The default table-load pass maps Exp->exp_and_others and
    # Ln->natural_log, inserting a ~2.7us table swap before nearly every
    # activation. Restrict the pass to the one set that covers both.
    def _patched_insert_act_table_loads(self):
        has_activation = any(
            isinstance(i, mybir.InstActivation)
            for b in self.main_func.blocks for i in b.instructions)
        if not has_activation:
            return
        keep = "natural_log_exp_and_others"
        tables = [(n, (s if n == keep else set()))
                  for n, s in get_activation_tables(self.m.arch).items()]
        _bass_rust.insert_act_table_loads(self, tables)

    nc.insert_act_table_loads = types.MethodType(
        _patched_insert_act_table_loads, nc)

    nc.compile()
    return nc


def _get_nc(repeat=1):
    key = ("nc", repeat)
    if key not in _CACHE:
        _CACHE[key] = _build_nc(repeat)
    return _CACHE[key]


def make_in_maps(feature, target, head_w, head_b):
    """Host-side sharding: returns per-core input dicts."""
    feature = np.ascontiguousarray(np.asarray(feature, dtype=np.float32))
    target = np.asarray(target, dtype=np.float32)
    head_w = np.asarray(head_w, dtype=np.float32)
    head_b = np.asarray(head_b, dtype=np.float32)

    # featT tiles: [NTILES, P(p=h%128... p is h within k), KT, P(tokens)]
    # featT_tiles[i, p, k, f] = feature_flat[i*128+f, k*128+p]
    ff = feature.reshape(TOK, H)
    featT_tiles = np.ascontiguousarray(
        ff.reshape(NTILES, P, KT, P).transpose(0, 3, 2, 1))

    tflat = target.reshape(TOK, T)
    in_maps = []
    for c in range(NCORES):
        hw_c = head_w[c * TC:(c + 1) * TC, :]       # [TC, H]
        wT_c = np.ascontiguousarray(
            hw_c.reshape(TC, KT, P).transpose(2, 1, 0))  # [P, KT, TC]
        b_c = np.ascontiguousarray(head_b[c * TC:(c + 1) * TC].reshape(1, TC))
        tgt_c = np.ascontiguousarray(tflat[:, c * TC:(c + 1) * TC])
        in_maps.append({"featT": featT_tiles, "wT": wT_c, "bias": b_c,
                        "tgt": tgt_c})
    return in_maps


def combine_outputs(results, target, mask, alpha):
    """Host-side gather: assemble logits + scalar loss from per-core outs."""
    target = np.asarray(target, dtype=np.float32)
    mask_f = np.asarray(mask).reshape(TOK).astype(np.float64)
    alpha = float(np.asarray(alpha))

    logits = np.empty((TOK, T), dtype=np.float32)
    r1 = np.zeros(TOK, dtype=np.float64)
    r2 = np.zeros(TOK, dtype=np.float64)
    r3 = np.zeros(TOK, dtype=np.float64)
    for c in range(NCORES):
        logits[:, c * TC:(c + 1) * TC] = results[c]["logits"]
        racc = results[c]["racc"].astype(np.float64)  # [P, 3*NTILES]
        # column i holds tile i's [128] partial; token = i*128 + p
        r1 += racc[:, 0:NTILES].T.reshape(TOK)
        r2 += racc[:, NTILES:2 * NTILES].T.reshape(TOK)
        r3 += racc[:, 2 * NTILES:3 * NTILES].T.reshape(TOK)

    rowsum_x = logits.sum(axis=1, dtype=np.float64)
    S1 = float(np.dot(mask_f, rowsum_x + r1))   # sum softplus(x)
    S2 = float(np.dot(mask_f, r2))              # sum x*target
    S3 = float(np.dot(mask_f, r3))              # sum x*sigmoid(prev), l>=1
    # l == 0 rows: prev = target[:, 0]
    lg = logits.reshape(B, L, T)
    tg = target.reshape(B, L, T)
    for b in range(B):
        S3 += mask_f[b * L] * float(
            np.dot(lg[b, 0].astype(np.float64), tg[b, 0].astype(np.float64)))

    num = S1 - (1.0 - alpha) * S2 - alpha * S3
    denom = max(mask_f.sum(), 1.0)
    loss = np.float32(num / denom)
    return lg, loss


def kernel(feature, target, mask, head_w, head_b, alpha):
    nc = _get_nc()
    in_maps = make_in_maps(feature, target, head_w, head_b)
    res = run_bass_kernel_spmd(nc, in_maps, list(range(NCORES)))
    logits, loss = combine_outputs(res.results, target, mask, alpha)
    return logits, loss


if __name__ == "__main__":
    # quick self-run with random data
    rng = np.random.default_rng(0)
    feature = rng.standard_normal((B, L, H), dtype=np.float32)
    target = rng.random((B, L, T), dtype=np.float32)
    mask = np.ones((B, L), dtype=bool)
    head_w = (rng.standard_normal((T, H), dtype=np.float32) / np.sqrt(H)).astype(np.float32)
    head_b = np.zeros((T,), dtype=np.float32)
    alpha = np.float32(0.37)
    lg, loss = kernel(feature=feature, target=target, mask=mask,
                      head_w=head_w, head_b=head_b, alpha=alpha)
    print("logits", lg.shape, lg.dtype, "loss", loss)


# revision 21
# speedup vs baseline: 3.0567x; 1.5802x over previous
"""Trainium2 Bass kernel for nn_AuxEntTypeTower (BCE-with-logits aux loss).

Computes, given feature [4,1024,512], target [4,1024,8192], mask [4,1024],
head_w [8192,512], head_b [8192], alpha []:
    logits = feature @ head_w.T + head_b                      # [4,1024,8192]
    prev   = concat([target[:, :1], sigmoid(logits[:, :-1])], axis=1)
    st     = (1-alpha)*target + alpha*prev
    per    = softplus(logits) - logits*st
    loss   = sum(per * mask[...,None]) / max(sum(mask), 1)
Returns (logits, loss).

Sharding: tensor-parallel over the type dim T=8192 across 8 cores (1024
types per core). Each core computes its logits chunk plus three per-token
partial reductions; the host combines them into the scalar loss:
    r1[tok] = sum_t softplus(-x)          (softplus(x) = x + softplus(-x))
    r2[tok] = sum_t x*target
    r3[tok] = sum_t x*sigmoid(x_prev_row) (prev-row shift, 0 at batch starts)
Device transcendentals use only the natural_log_exp table set:
    en = exp(-x); spn = ln(1+en) = softplus(-x); sg = exp(-spn) = sigmoid(x).
"""

import types

import numpy as np

import bass_rust as _bass_rust
import concourse.bacc as bacc
import concourse.bass as bass
import concourse.mybir as mybir
import concourse.tile as tile
from concourse.bass_utils import run_bass_kernel_spmd
from concourse.hw_specs import get_activation_tables

B, L, H, T = 4, 1024, 512, 8192
NCORES = 8
TC = T // NCORES          # 1024 types per core
TOK = B * L               # 4096 tokens
P = 128                   # partitions per tile
NTILES = TOK // P         # 32 token tiles per core
KT = H // P               # 4 contraction sub-tiles
NH = TC // 512            # 2 matmul n-halves per tile
BATCH_TILES = L // P      # 8 tiles per batch sequence

F32 = mybir.dt.float32
F32R = mybir.dt.float32r

_CACHE = {}


def _build_nc(repeat=1):
    """repeat>1 re-runs the whole pipeline (for marginal-cost timing)."""
    nc = bacc.Bacc("TRN2", target_bir_lowering=False, debug=False,
                   num_devices=NCORES)

    featT = nc.declare_dram_parameter("featT", [NTILES, P, KT, P], F32,
                                      isOutput=False)
    wT = nc.declare_dram_parameter("wT", [P, KT, TC], F32, isOutput=False)
    bias = nc.declare_dram_parameter("bias", [1, TC], F32, isOutput=False)
    tgt = nc.declare_dram_parameter("tgt", [TOK, TC], F32, isOutput=False)
    logits = nc.declare_dram_parameter("logits", [TOK, TC], F32, isOutput=True)
    racc_d = nc.declare_dram_parameter("racc", [P, 3 * NTILES], F32,
                                       isOutput=True)

    AF = mybir.ActivationFunctionType
    OP = mybir.AluOpType

    with tile.TileContext(nc) as tc:
        # shift matrices: S moves row p -> p+1 within a tile (superdiagonal),
        # E moves prev-tile row 127 -> row 0 (for the cross-tile boundary)
        s_np = np.zeros((P, P), np.float32)
        for m in range(1, P):
            s_np[m - 1, m] = 1.0
        e_np = np.zeros((P, P), np.float32)
        e_np[P - 1, 0] = 1.0
        s_dram = nc.inline_tensor(s_np, name="shiftS")
        e_dram = nc.inline_tensor(e_np, name="shiftE")

        with (
            tc.tile_pool(name="consts", bufs=1) as consts,
            tc.tile_pool(name="feat", bufs=4) as featp,
            tc.tile_pool(name="featr", bufs=4) as featrp,
            tc.tile_pool(name="tgtp", bufs=4) as tgtp,
            tc.tile_pool(name="xps", bufs=2, space="PSUM") as xps,
            tc.tile_pool(name="sgps", bufs=2, space="PSUM") as sgps,
            tc.tile_pool(name="xsb", bufs=3) as xsbp,
            tc.tile_pool(name="en", bufs=2) as enp,
            tc.tile_pool(name="spn", bufs=2) as spnp,
            tc.tile_pool(name="sg", bufs=3) as sgp,
            tc.tile_pool(name="trash", bufs=2) as trashp,
        ):
            s_f32 = consts.tile([P, P], F32)
            nc.sync.dma_start(out=s_f32, in_=s_dram[:])
            s_r = consts.tile([P, P], F32R)
            nc.gpsimd.tensor_copy(out=s_r, in_=s_f32)
            e_f32 = consts.tile([P, P], F32)
            nc.sync.dma_start(out=e_f32, in_=e_dram[:])
            e_r = consts.tile([P, P], F32R)
            nc.gpsimd.tensor_copy(out=e_r, in_=e_f32)
            # fp32r operands: HWDGE loads raw fp32, a gpsimd copy rounds to
            # fp32r (SWDGE cast-DMAs are much slower than HWDGE + engine cast;
            # gpsimd keeps the cast off the busy DVE)
            wT_f32 = consts.tile([P, KT, TC], F32)
            nc.sync.dma_start(out=wT_f32, in_=wT[:])
            wT_sb = consts.tile([P, KT, TC], F32R)
            nc.gpsimd.tensor_copy(out=wT_sb, in_=wT_f32)
            # head_b broadcast across all 128 partitions (exact fp32 bias add)
            bias_sb = consts.tile([P, TC], F32)
            bias_ap = bias[:]
            bias_bcast = bass.AP(tensor=bias_ap.tensor, offset=bias_ap.offset,
                                 ap=[[0, P], [1, TC]])
            nc.sync.dma_start(out=bias_sb, in_=bias_bcast)
            racc = consts.tile([P, 3 * NTILES], F32)

            for _rep in range(repeat):
              sg_prev = None
              for i in range(NTILES):
                feat_f32 = featp.tile([P, KT, P], F32)
                nc.sync.dma_start(out=feat_f32, in_=featT[i])
                feat_t = featrp.tile([P, KT, P], F32R)
                nc.gpsimd.tensor_copy(out=feat_t, in_=feat_f32)
                tgt_t = tgtp.tile([P, TC], F32)
                nc.sync.dma_start(out=tgt_t, in_=tgt[i * P:(i + 1) * P, :])

                x_ps = xps.tile([P, TC], F32)
                for nh in range(NH):
                    csl = bass.ts(nh, 512)
                    for k in range(KT):
                        nc.tensor.matmul(
                            out=x_ps[:, csl],
                            lhsT=feat_t[:, k, :],
                            rhs=wT_sb[:, k, csl],
                            start=(k == 0),
                            stop=(k == KT - 1),
                        )

                # x = psum + bias  (exact fp32), lands in SBUF for DMA/reads
                x_sb = xsbp.tile([P, TC], F32)
                nc.vector.tensor_add(out=x_sb, in0=x_ps, in1=bias_sb)
                nc.sync.dma_start(out=logits[i * P:(i + 1) * P, :], in_=x_sb)

                # ACT chain (single table set): en=exp(-x), spn=ln(1+en),
                # sg=exp(-spn)=sigmoid(x). accum(spn) -> r1 column.
                en_t = enp.tile([P, TC], F32)
                nc.scalar.activation(out=en_t, in_=x_sb, func=AF.Exp,
                                     scale=-1.0)
                spn_t = spnp.tile([P, TC], F32)
                nc.scalar.activation(out=spn_t, in_=en_t, func=AF.Ln,
                                     bias=1.0, accum_out=racc[:, i:i + 1])
                # sigmoid, written as fp32r so the PE can consume it
                sg_t = sgp.tile([P, TC], F32R)
                nc.scalar.activation(out=sg_t, in_=spn_t, func=AF.Exp,
                                     scale=-1.0)

                # prev-row shift of sigmoid on the PE: sgs = S.T@sg (+E.T@prev)
                sgs_t = sgps.tile([P, TC], F32)
                first = i % BATCH_TILES == 0
                for nh in range(NH):
                    csl = bass.ts(nh, 512)
                    nc.tensor.matmul(out=sgs_t[:, csl], lhsT=s_r,
                                     rhs=sg_t[:, csl],
                                     start=True, stop=first)
                    if not first:
                        nc.tensor.matmul(out=sgs_t[:, csl], lhsT=e_r,
                                         rhs=sg_prev[:, csl],
                                         start=False, stop=True)
                sg_prev = sg_t

                # fused multiply+reduce: r2 = sum x*t, r3 = sum x*sg_shift
                # (scalar_tensor_tensor: out=(in0*1.0)*in1, accum=sum(out))
                tr = trashp.tile([P, TC], F32)
                nc.vector.scalar_tensor_tensor(
                    out=tr, in0=x_sb, scalar=1.0, in1=tgt_t,
                    op0=OP.mult, op1=OP.mult,
                    accum_out=racc[:, NTILES + i:NTILES + i + 1])
                tr2 = trashp.tile([P, TC], F32)
                nc.vector.scalar_tensor_tensor(
                    out=tr2, in0=x_sb, scalar=1.0, in1=sgs_t,
                    op0=OP.mult, op1=OP.mult,
                    accum_out=racc[:, 2 * NTILES + i:2 * NTILES + i + 1])

            nc.sync.dma_start(out=racc_d[:], in_=racc)

    # All our transcendentals (Exp, Ln) live in natural_log_exp_and_others.
    # The default table-load pass maps Exp->exp_and_others and
    # Ln->natural_log, inserting a ~2.7us table swap before nearly every
    # activation. Restrict the pass to the one set that covers both.
    def _patched_insert_act_table_loads(self):
        has_activation = any(
            isinstance(i, mybir.InstActivation)
            for b in self.main_func.blocks for i in b.instructions)
        if not has_activation:
            return
        keep = "natural_log_exp_and_others"
        tables = [(n, (s if n == keep else set()))
                  for n, s in get_activation_tables(self.m.arch).items()]
        _bass_rust.insert_act_table_loads(self, tables)

    nc.insert_act_table_loads = types.MethodType(
        _patched_insert_act_table_loads, nc)

    nc.compile()
    return nc


def _get_nc(repeat=1):
    key = ("nc", repeat)
    if key not in _CACHE:
        _CACHE[key] = _build_nc(repeat)
    return _CACHE[key]


def make_in_maps(feature, target, head_w, head_b):
    """Host-side sharding: returns per-core input dicts."""
    feature = np.ascontiguousarray(np.asarray(feature, dtype=np.float32))
    target = np.asarray(target, dtype=np.float32)
    head_w = np.asarray(head_w, dtype=np.float32)
    head_b = np.asarray(head_b, dtype=np.float32)

    # featT tiles: [NTILES, P(p=h%128... p is h within k), KT, P(tokens)]
    # featT_tiles[i, p, k, f] = feature_flat[i*128+f, k*128+p]
    ff = feature.reshape(TOK, H)
    featT_tiles = np.ascontiguousarray(
        ff.reshape(NTILES, P, KT, P).transpose(0, 3, 2, 1))

    tflat = target.reshape(TOK, T)
    in_maps = []
    for c in range(NCORES):
        hw_c = head_w[c * TC:(c + 1) * TC, :]       # [TC, H]
        wT_c = np.ascontiguousarray(
            hw_c.reshape(TC, KT, P).transpose(2, 1, 0))  # [P, KT, TC]
        b_c = np.ascontiguousarray(head_b[c * TC:(c + 1) * TC].reshape(1, TC))
        tgt_c = np.ascontiguousarray(tflat[:, c * TC:(c + 1) * TC])
        in_maps.append({"featT": featT_tiles, "wT": wT_c, "bias": b_c,
                        "tgt": tgt_c})
    return in_maps


def combine_outputs(results, target, mask, alpha):
    """Host-side gather: assemble logits + scalar loss from per-core outs."""
    target = np.asarray(target, dtype=np.float32)
    mask_f = np.asarray(mask).reshape(TOK).astype(np.float64)
    alpha = float(np.asarray(alpha))

    logits = np.empty((TOK, T), dtype=np.float32)
    r1 = np.zeros(TOK, dtype=np.float64)
    r2 = np.zeros(TOK, dtype=np.float64)
    r3 = np.zeros(TOK, dtype=np.float64)
    for c in range(NCORES):
        logits[:, c * TC:(c + 1) * TC] = results[c]["logits"]
        racc = results[c]["racc"].astype(np.float64)  # [P, 3*NTILES]
        # column i holds tile i's [128] partial; token = i*128 + p
        r1 += racc[:, 0:NTILES].T.reshape(TOK)
        r2 += racc[:, NTILES:2 * NTILES].T.reshape(TOK)
        r3 += racc[:, 2 * NTILES:3 * NTILES].T.reshape(TOK)

    rowsum_x = logits.sum(axis=1, dtype=np.float64)
    S1 = float(np.dot(mask_f, rowsum_x + r1))   # sum softplus(x)
    S2 = float(np.dot(mask_f, r2))              # sum x*target
    S3 = float(np.dot(mask_f, r3))              # sum x*sigmoid(prev), l>=1
    # l == 0 rows: prev = target[:, 0]
    lg = logits.reshape(B, L, T)
    tg = target.reshape(B, L, T)
    for b in range(B):
        S3 += mask_f[b * L] * float(
            np.dot(lg[b, 0].astype(np.float64), tg[b, 0].astype(np.float64)))

    num = S1 - (1.0 - alpha) * S2 - alpha * S3
    denom = max(mask_f.sum(), 1.0)
    loss = np.float32(num / denom)
    return lg, loss


def kernel(feature, target, mask, head_w, head_b, alpha):
    nc = _get_nc()
    in_maps = make_in_maps(feature, target, head_w, head_b)
    res = run_bass_kernel_spmd(nc, in_maps, list(range(NCORES)))
    logits, loss = combine_outputs(res.results, target, mask, alpha)
    return logits, loss


if __name__ == "__main__":
    # quick self-run with random data
    rng = np.random.default_rng(0)
    feature = rng.standard_normal((B, L, H), dtype=np.float32)
    target = rng.random((B, L, T), dtype=np.float32)
    mask = np.ones((B, L), dtype=bool)
    head_w = (rng.standard_normal((T, H), dtype=np.float32) / np.sqrt(H)).astype(np.float32)
    head_b = np.zeros((T,), dtype=np.float32)
    alpha = np.float32(0.37)
    lg, loss = kernel(feature=feature, target=target, mask=mask,
                      head_w=head_w, head_b=head_b, alpha=alpha)
    print("logits", lg.shape, lg.dtype, "loss", loss)


# revision 23
# speedup vs baseline: 6.4519x; 2.1107x over previous
"""Trainium2 Bass kernel for nn_AuxEntTypeTower (BCE-with-logits aux loss).

Computes, given feature [4,1024,512], target [4,1024,8192], mask [4,1024],
head_w [8192,512], head_b [8192], alpha []:
    logits = feature @ head_w.T + head_b                      # [4,1024,8192]
    prev   = concat([target[:, :1], sigmoid(logits[:, :-1])], axis=1)
    st     = (1-alpha)*target + alpha*prev
    per    = softplus(logits) - logits*st
    loss   = sum(per * mask[...,None]) / max(sum(mask), 1)
Returns (logits, loss).

Sharding: tensor-parallel over the type dim T=8192 across 8 cores (1024
types per core). Each core computes its logits chunk plus three per-token
partial reductions; the host combines them into the scalar loss:
    r1[tok] = sum_t softplus(-x)          (softplus(x) = x + softplus(-x))
    r2[tok] = sum_t x*target
    r3[tok] = sum_t x*sigmoid(x_prev_row) (prev-row shift, 0 at batch starts)
Device transcendentals use only the natural_log_exp table set:
    en = exp(-x); spn = ln(1+en) = softplus(-x); sg = exp(-spn) = sigmoid(x).
"""

import types

import numpy as np

import bass_rust as _bass_rust
import concourse.bacc as bacc
import concourse.bass as bass
import concourse.mybir as mybir
import concourse.tile as tile
from concourse.bass_utils import run_bass_kernel_spmd
from concourse.hw_specs import get_activation_tables

B, L, H, T = 4, 1024, 512, 8192
NCORES = 8
TC = T // NCORES          # 1024 types per core
TOK = B * L               # 4096 tokens
P = 128                   # partitions per tile
NTILES = TOK // P         # 32 token tiles per core
KT = H // P               # 4 contraction sub-tiles
NH = TC // 512            # 2 matmul n-halves per tile
BATCH_TILES = L // P      # 8 tiles per batch sequence

F32 = mybir.dt.float32
F32R = mybir.dt.float32r

_CACHE = {}


def _build_nc(repeat=1):
    """repeat>1 re-runs the whole pipeline (for marginal-cost timing)."""
    nc = bacc.Bacc("TRN2", target_bir_lowering=False, debug=False,
                   num_devices=NCORES)

    featT = nc.declare_dram_parameter("featT", [NTILES, P, KT, P], F32,
                                      isOutput=False)
    wT = nc.declare_dram_parameter("wT", [P, KT, TC], F32, isOutput=False)
    bias = nc.declare_dram_parameter("bias", [1, TC], F32, isOutput=False)
    tgt = nc.declare_dram_parameter("tgt", [TOK, TC], F32, isOutput=False)
    logits = nc.declare_dram_parameter("logits", [TOK, TC], F32, isOutput=True)
    racc_d = nc.declare_dram_parameter("racc", [P, 3 * NTILES], F32,
                                       isOutput=True)

    AF = mybir.ActivationFunctionType
    OP = mybir.AluOpType

    with tile.TileContext(nc) as tc:
        # shift matrices: S moves row p -> p+1 within a tile (superdiagonal),
        # E moves prev-tile row 127 -> row 0 (for the cross-tile boundary)
        s_np = np.zeros((P, P), np.float32)
        for m in range(1, P):
            s_np[m - 1, m] = 1.0
        e_np = np.zeros((P, P), np.float32)
        e_np[P - 1, 0] = 1.0
        s_dram = nc.inline_tensor(s_np, name="shiftS")
        e_dram = nc.inline_tensor(e_np, name="shiftE")

        with (
            tc.tile_pool(name="consts", bufs=1) as consts,
            tc.tile_pool(name="feat", bufs=4) as featp,
            tc.tile_pool(name="featr", bufs=4) as featrp,
            tc.tile_pool(name="tgtp", bufs=4) as tgtp,
            tc.tile_pool(name="xps", bufs=2, space="PSUM") as xps,
            tc.tile_pool(name="sgps", bufs=2, space="PSUM") as sgps,
            tc.tile_pool(name="xsb", bufs=2) as xsbp,
            tc.tile_pool(name="en", bufs=2) as enp,
            tc.tile_pool(name="spn", bufs=2) as spnp,
            tc.tile_pool(name="sg", bufs=3) as sgp,
            tc.tile_pool(name="trash", bufs=2) as trashp,
        ):
            s_f32 = consts.tile([P, P], F32)
            nc.sync.dma_start(out=s_f32, in_=s_dram[:])
            s_r = consts.tile([P, P], F32R)
            nc.gpsimd.tensor_copy(out=s_r, in_=s_f32)
            e_f32 = consts.tile([P, P], F32)
            nc.sync.dma_start(out=e_f32, in_=e_dram[:])
            e_r = consts.tile([P, P], F32R)
            nc.gpsimd.tensor_copy(out=e_r, in_=e_f32)
            # fp32r operands: HWDGE loads raw fp32, a gpsimd copy rounds to
            # fp32r (SWDGE cast-DMAs are much slower than HWDGE + engine cast;
            # gpsimd keeps the cast off the busy DVE)
            wT_f32 = consts.tile([P, KT, TC], F32)
            nc.sync.dma_start(out=wT_f32, in_=wT[:])
            wT_sb = consts.tile([P, KT, TC], F32R)
            nc.gpsimd.tensor_copy(out=wT_sb, in_=wT_f32)
            # head_b broadcast across all 128 partitions (exact fp32 bias add)
            bias_sb = consts.tile([P, TC], F32)
            bias_ap = bias[:]
            bias_bcast = bass.AP(tensor=bias_ap.tensor, offset=bias_ap.offset,
                                 ap=[[0, P], [1, TC]])
            nc.sync.dma_start(out=bias_sb, in_=bias_bcast)
            racc = consts.tile([P, 3 * NTILES], F32)

            for _rep in range(repeat):
              sg_prev = None
              # process token-tiles in pairs: the two Exp passes run as one
              # [P, 2*TC] instruction per pair (amortizes the ~352-cycle ACT
              # per-instruction overhead); Ln stays per-tile for its accum.
              for pair in range(NTILES // 2):
                x_pair = xsbp.tile([P, 2, TC], F32)
                tgt_views = []
                for j in range(2):
                    i = 2 * pair + j
                    feat_f32 = featp.tile([P, KT, P], F32)
                    nc.sync.dma_start(out=feat_f32, in_=featT[i])
                    feat_t = featrp.tile([P, KT, P], F32R)
                    nc.gpsimd.tensor_copy(out=feat_t, in_=feat_f32)
                    tgt_t = tgtp.tile([P, TC], F32)
                    nc.sync.dma_start(out=tgt_t, in_=tgt[i * P:(i + 1) * P, :])
                    tgt_views.append(tgt_t)

                    x_ps = xps.tile([P, TC], F32)
                    for k in range(KT):
                        for nh in range(NH):
                            csl = bass.ts(nh, 512)
                            nc.tensor.matmul(
                                out=x_ps[:, csl],
                                lhsT=feat_t[:, k, :],
                                rhs=wT_sb[:, k, csl],
                                start=(k == 0),
                                stop=(k == KT - 1),
                                skip_group_check=True,
                            )

                    # x = psum + bias (exact fp32), lands in SBUF
                    nc.vector.tensor_add(out=x_pair[:, j, :], in0=x_ps,
                                         in1=bias_sb)
                    nc.sync.dma_start(out=logits[i * P:(i + 1) * P, :],
                                      in_=x_pair[:, j, :])
                    # r2 = sum x*t (independent of the ACT chain)
                    tr = trashp.tile([P, TC], F32)
                    nc.vector.scalar_tensor_tensor(
                        out=tr, in0=x_pair[:, j, :], scalar=1.0, in1=tgt_t,
                        op0=OP.mult, op1=OP.mult,
                        accum_out=racc[:, NTILES + i:NTILES + i + 1])

                # ACT chain (single table set): en=exp(-x), spn=ln(1+en),
                # sg=exp(-spn)=sigmoid(x). accum(spn) -> r1 columns.
                en_t = enp.tile([P, 2, TC], F32)
                nc.scalar.activation(out=en_t, in_=x_pair, func=AF.Exp,
                                     scale=-1.0)
                spn_t = spnp.tile([P, 2, TC], F32)
                for j in range(2):
                    i = 2 * pair + j
                    nc.scalar.activation(out=spn_t[:, j, :],
                                         in_=en_t[:, j, :], func=AF.Ln,
                                         bias=1.0,
                                         accum_out=racc[:, i:i + 1])
                # sigmoid, written as fp32r so the PE can consume it
                sg_t = sgp.tile([P, 2, TC], F32R)
                nc.scalar.activation(out=sg_t, in_=spn_t, func=AF.Exp,
                                     scale=-1.0)

                for j in range(2):
                    i = 2 * pair + j
                    sg_cur = sg_t[:, j, :]
                    # prev-row shift of sigmoid on the PE:
                    # sgs = S.T@sg (+ E.T@prev for the cross-tile boundary)
                    sgs_t = sgps.tile([P, TC], F32)
                    first = i % BATCH_TILES == 0
                    for nh in range(NH):
                        csl = bass.ts(nh, 512)
                        nc.tensor.matmul(out=sgs_t[:, csl], lhsT=s_r,
                                         rhs=sg_cur[:, csl],
                                         start=True, stop=first)
                        if not first:
                            nc.tensor.matmul(out=sgs_t[:, csl], lhsT=e_r,
                                             rhs=sg_prev[:, csl],
                                             start=False, stop=True)
                    sg_prev = sg_cur

                    # r3 = sum x*sg_shift
                    tr2 = trashp.tile([P, TC], F32)
                    nc.vector.scalar_tensor_tensor(
                        out=tr2, in0=x_pair[:, j, :], scalar=1.0, in1=sgs_t,
                        op0=OP.mult, op1=OP.mult,
                        accum_out=racc[:, 2 * NTILES + i:2 * NTILES + i + 1])

            nc.sync.dma_start(out=racc_d[:], in_=racc)

    # All our transcendentals (Exp, Ln) live in natural_log_exp_and_others.
    # The default table-load pass maps Exp->exp_and_others and
    # Ln->natural_log, inserting a ~2.7us table swap before nearly every
    # activation. Restrict the pass to the one set that covers both.
    def _patched_insert_act_table_loads(self):
        has_activation = any(
            isinstance(i, mybir.InstActivation)
            for b in self.main_func.blocks for i in b.instructions)
        if not has_activation:
            return
        keep = "natural_log_exp_and_others"
        tables = [(n, (s if n == keep else set()))
                  for n, s in get_activation_tables(self.m.arch).items()]
        _bass_rust.insert_act_table_loads(self, tables)

    nc.insert_act_table_loads = types.MethodType(
        _patched_insert_act_table_loads, nc)

    nc.compile()
    return nc


def _get_nc(repeat=1):
    key = ("nc", repeat)
    if key not in _CACHE:
        _CACHE[key] = _build_nc(repeat)
    return _CACHE[key]


def make_in_maps(feature, target, head_w, head_b):
    """Host-side sharding: returns per-core input dicts."""
    feature = np.ascontiguousarray(np.asarray(feature, dtype=np.float32))
    target = np.asarray(target, dtype=np.float32)
    head_w = np.asarray(head_w, dtype=np.float32)
    head_b = np.asarray(head_b, dtype=np.float32)

    # featT tiles: [NTILES, P(p=h%128... p is h within k), KT, P(tokens)]
    # featT_tiles[i, p, k, f] = feature_flat[i*128+f, k*128+p]
    ff = feature.reshape(TOK, H)
    featT_tiles = np.ascontiguousarray(
        ff.reshape(NTILES, P, KT, P).transpose(0, 3, 2, 1))

    tflat = target.reshape(TOK, T)
    in_maps = []
    for c in range(NCORES):
        hw_c = head_w[c * TC:(c + 1) * TC, :]       # [TC, H]
        wT_c = np.ascontiguousarray(
            hw_c.reshape(TC, KT, P).transpose(2, 1, 0))  # [P, KT, TC]
        b_c = np.ascontiguousarray(head_b[c * TC:(c + 1) * TC].reshape(1, TC))
        tgt_c = np.ascontiguousarray(tflat[:, c * TC:(c + 1) * TC])
        in_maps.append({"featT": featT_tiles, "wT": wT_c, "bias": b_c,
                        "tgt": tgt_c})
    return in_maps


def combine_outputs(results, target, mask, alpha):
    """Host-side gather: assemble logits + scalar loss from per-core outs."""
    target = np.asarray(target, dtype=np.float32)
    mask_f = np.asarray(mask).reshape(TOK).astype(np.float64)
    alpha = float(np.asarray(alpha))

    logits = np.empty((TOK, T), dtype=np.float32)
    r1 = np.zeros(TOK, dtype=np.float64)
    r2 = np.zeros(TOK, dtype=np.float64)
    r3 = np.zeros(TOK, dtype=np.float64)
    for c in range(NCORES):
        logits[:, c * TC:(c + 1) * TC] = results[c]["logits"]
        racc = results[c]["racc"].astype(np.float64)  # [P, 3*NTILES]
        # column i holds tile i's [128] partial; token = i*128 + p
        r1 += racc[:, 0:NTILES].T.reshape(TOK)
        r2 += racc[:, NTILES:2 * NTILES].T.reshape(TOK)
        r3 += racc[:, 2 * NTILES:3 * NTILES].T.reshape(TOK)

    rowsum_x = logits.sum(axis=1, dtype=np.float64)
    S1 = float(np.dot(mask_f, rowsum_x + r1))   # sum softplus(x)
    S2 = float(np.dot(mask_f, r2))              # sum x*target
    S3 = float(np.dot(mask_f, r3))              # sum x*sigmoid(prev), l>=1
    # l == 0 rows: prev = target[:, 0]
    lg = logits.reshape(B, L, T)
    tg = target.reshape(B, L, T)
    for b in range(B):
        S3 += mask_f[b * L] * float(
            np.dot(lg[b, 0].astype(np.float64), tg[b, 0].astype(np.float64)))

    num = S1 - (1.0 - alpha) * S2 - alpha * S3
    denom = max(mask_f.sum(), 1.0)
    loss = np.float32(num / denom)
    return lg, loss


def kernel(feature, target, mask, head_w, head_b, alpha):
    nc = _get_nc()
    in_maps = make_in_maps(feature, target, head_w, head_b)
    res = run_bass_kernel_spmd(nc, in_maps, list(range(NCORES)))
    logits, loss = combine_outputs(res.results, target, mask, alpha)
    return logits, loss


if __name__ == "__main__":
    # quick self-run with random data
    rng = np.random.default_rng(0)
    feature = rng.standard_normal((B, L, H), dtype=np.float32)
    target = rng.random((B, L, T), dtype=np.float32)
    mask = np.ones((B, L), dtype=bool)
    head_w = (rng.standard_normal((T, H), dtype=np.float32) / np.sqrt(H)).astype(np.float32)
    head_b = np.zeros((T,), dtype=np.float32)
    alpha = np.float32(0.37)
    lg, loss = kernel(feature=feature, target=target, mask=mask,
                      head_w=head_w, head_b=head_b, alpha=alpha)
    print("logits", lg.shape, lg.dtype, "loss", loss)
